# revision 17
# baseline (speedup 1.0000x reference)
"""AlphaModel (relation-gated message passing) Trainium2 kernel, v3.

Strategy (pure data parallel, per sharding hint):
  - Shard the 8M edges across 8 NeuronCores (1M each).
  - Host SORTS each core's edges by relation id and packs them into
    (tile, partition) cells so that every SBUF partition row processes
    edges of a single relation.  M[rel] / beta[rel] then enter the
    device as per-partition scalar vectors [128,1] (fp32), so the
    relation-gated matvec and beta-mix become tensor_scalar ops (DVE
    2x/4x perf modes) instead of streamed per-edge tables.
  - fp16 end-to-end on-chip (2e-2 rel tolerance).  tensor_tensor ops
    hit the 2x_1p DVE mode, tensor_scalar the 4x_2p mode.
  - The c- and p-branch sparsemaxes are FUSED: both live in one wide
    [128, 6B] tile ([c0 c1 c2 | p0 p1 p2]) and the whole max/min/sum
    tree + tau runs as [128, 2, B] strided ops covering both branches.
  - Activation-table thrash eliminated: only Ln / Square / one Exp on
    ACT (grouped), reciprocals on the DVE (nc.vector.reciprocal).
  - sparsemax (d=3) via simplex projection:
      tau = max(mx-1, (sm-mn-1)/2, (sm-1)/3);  out = relu(x - tau)
    The second sparsemax application in the reference is an exact no-op
    (idempotent projection) and is skipped.

Output: alpha [8M, 3] float32 (device emits fp16, host widens).
"""

import sys

if "/opt/trn_rl_repo" not in sys.path:
    sys.path.insert(0, "/opt/trn_rl_repo")

import numpy as np

import concourse.bacc as bacc
import concourse.mybir as mybir
from concourse.bass_utils import run_bass_kernel_spmd
from concourse.tile import TileContext

N_CORES = 8
PDIM = 128
N_RELS = 64
B = 1152  # edges per (tile, partition) cell

AF = mybir.ActivationFunctionType
OP = mybir.AluOpType
F16 = mybir.dt.float16
F32 = mybir.dt.float32

# --------------------------------------------------------------------------
# Custom fused DVE op (same registration machinery as production dve_ops).
# --------------------------------------------------------------------------
_OPS_CACHE: dict = {}


def _custom_ops():
    if _OPS_CACHE:
        return _OPS_CACHE
    from concourse import dve_ops
    from concourse.dve_ops import DveOp, OPS, _SUB_OPCODE_FOR_NAME
    from concourse.dve_spec import (
        C0,
        C1,
        One,
        Spec,
        Src0,
        Src1,
        _has_src1,
        lower,
        maxx,
    )
    from concourse.dve_uop import DveOpSpec

    existing = {op.name: op for op in OPS}

    def mk(key, name, body):
        if name in existing:
            _OPS_CACHE[key] = existing[name]
            return
        if name not in _SUB_OPCODE_FOR_NAME:
            row = max(_SUB_OPCODE_FOR_NAME.values()) + 1
            assert row < 0x20, "custom DVE opcode rows exhausted"
            _SUB_OPCODE_FOR_NAME[name] = row
        spec = Spec(body=body)
        shas = {}
        for ver in ("v3", "v4"):
            uops = lower(spec, ver=ver)
            s = DveOpSpec(
                name=name,
                opcode=_SUB_OPCODE_FOR_NAME[name],
                uops=uops,
                rd1_en=_has_src1(spec),
            )
            shas[ver] = s.sha(ver)
        op = DveOp(name, spec, subdim=False, uops_sha=shas)
        OPS.append(op)
        dve_ops.CUSTOM_DVE_SPECS[name] = spec
        _OPS_CACHE[key] = op

    # tau candidates: max((sm - mn - 1)*0.5, (sm - 1)/3);  in0=sm, in1=mn
    mk("tau_a", "ANT_TAU_A", maxx((Src0 - Src1 - One) * C0, (Src0 - One) * C1))
    # alpha0 = b*c + (1-b)*p with per-partition scalar APs s0=b, s1=(1-b)
    mk("aff2", "ANT_AFF2", Src0 * C0 + Src1 * C1)
    return _OPS_CACHE


# --------------------------------------------------------------------------
# Bass program
# --------------------------------------------------------------------------
_PROG_CACHE: dict = {}


def _build_program(z_eps: float, scale_factor: float, T: int):
    ops = _custom_ops()
    nc = bacc.Bacc(
        "TRN2",
        target_bir_lowering=False,
        num_devices=N_CORES,
        dynamic_dma_scratch_size=8192,
    )

    # Input stream per tile/partition: [ch0 ch1 ch2 | pp0 pp1 pp2] fp16
    xin_d = nc.dram_tensor("xin", [T * PDIM * 6 * B], F16, kind="ExternalInput")
    # Per (tile, partition) scalars: M00..M22, b0..b2, (1-b)0..(1-b)2, pad
    scl_d = nc.dram_tensor("scl", [T * PDIM * 16], F32, kind="ExternalInput")
    out_d = nc.dram_tensor("alpha", [T * PDIM * 3 * B], F16, kind="ExternalOutput")

    xin_v = xin_d[:].rearrange("(t p c) -> t p c", t=T, p=PDIM)
    scl_v = scl_d[:].rearrange("(t p c) -> t p c", t=T, p=PDIM)
    out_v = out_d[:].rearrange("(t p c) -> t p c", t=T, p=PDIM)

    V = nc.vector
    A = nc.scalar

    with TileContext(nc) as tc:
        with (
            nc.allow_low_precision(reason="fp16 pipeline; 2e-2 rel tolerance"),
            tc.tile_pool(name="io", bufs=2) as iop,
            tc.tile_pool(name="scr", bufs=2) as sp,
        ):
            for t in range(T):
                xin = iop.tile([PDIM, 6 * B], F16, tag="xin", name=f"xin{t}")
                scl = iop.tile([PDIM, 16], F32, tag="scl", name=f"scl{t}")
                ot = iop.tile([PDIM, 3 * B], F16, tag="ot", name=f"ot{t}")
                nc.sync.dma_start(xin[:], xin_v[t])
                nc.sync.dma_start(scl[:], scl_v[t])

                X = xin[:, 0 : 3 * B]  # child planes
                Pr = xin[:, 3 * B : 6 * B]  # parent (raw) planes

                def w6(tag, bufs=2):
                    return sp.tile(
                        [PDIM, 6 * B], F16, tag=tag, name=f"{tag}{t}", bufs=bufs
                    )[:]

                def w3(tag, bufs=1):
                    return sp.tile(
                        [PDIM, 3 * B], F16, tag=tag, name=f"{tag}{t}", bufs=bufs
                    )[:]

                def w2(tag, bufs=2):
                    return sp.tile(
                        [PDIM, 2 * B], F16, tag=tag, name=f"{tag}{t}", bufs=bufs
                    )[:]

                def pl(tag, bufs=1):
                    return sp.tile(
                        [PDIM, B], F16, tag=tag, name=f"{tag}{t}", bufs=bufs
                    )[:]

                def p3(x, i):
                    return x[:, i * B : (i + 1) * B]

                def sv(j):
                    return scl[:, j : j + 1]

                W = w6("W")  # [c_raw | p_raw] then later scratch
                SPX = w6("SPX")  # [c | p] sparsemax outputs
                YL = w6("YL", bufs=1)  # [y | y ln y]
                W9 = sp.tile([PDIM, 9 * B], F16, tag="W9", name=f"W9{t}", bufs=1)[:]

                CRw = W[:, 0 : 3 * B]
                # ---- c_raw_i = sum_j M_ij * ch_j: the 9 products run on the
                # ACT engine (Copy with per-partition scale) into W9 scratch;
                # the DVE only does the 6 accumulating adds.
                for i in range(3):
                    for j in range(3):
                        A.mul(p3(W9, 3 * i + j), p3(X, j), sv(3 * i + j))
                for i in range(3):
                    V.tensor_add(p3(CRw, i), p3(W9, 3 * i), p3(W9, 3 * i + 1))
                    V.tensor_add(p3(CRw, i), p3(CRw, i), p3(W9, 3 * i + 2))
                # copy p_raw next to c_raw (4x tensor_copy)
                V.tensor_copy(W[:, 3 * B : 6 * B], Pr)

                # ---- fused two-branch sparsemax: SPX = relu(W - tau)
                # strided views: component j of both branches = [128, 2, B]
                Wv = W.rearrange("p (u b) -> p u b", b=B)  # u = 6

                def comp(x6, j):
                    v = x6.rearrange("p (u b) -> p u b", b=B)
                    return v[:, j : j + 4 : 3, :]  # planes {j, j+3}

                mx = w2("mx")
                mn = w2("mn")
                sm = w2("sm")
                ta = w2("ta")
                mx2 = mx.rearrange("p (u b) -> p u b", b=B)
                mn2 = mn.rearrange("p (u b) -> p u b", b=B)
                sm2 = sm.rearrange("p (u b) -> p u b", b=B)
                ta2 = ta.rearrange("p (u b) -> p u b", b=B)
                x0, x1, x2 = comp(W, 0), comp(W, 1), comp(W, 2)
                V.tensor_max(mx2, x0, x1)
                V.tensor_tensor(mn2, x0, x1, OP.min)
                V.tensor_add(sm2, x0, x1)
                V.tensor_max(mx2, mx2, x2)
                V.tensor_tensor(mn2, mn2, x2, OP.min)
                V.tensor_add(sm2, sm2, x2)
                V._custom_dve(ops["tau_a"], out=ta, in0=sm, in1=mn, s0=0.5, s1=1.0 / 3.0)
                V.tensor_scalar(mx, mx, -1.0, None, OP.add)  # mx-1 (4x)
                V.tensor_max(ta, ta, mx)  # tau for both branches
                for j in range(3):
                    V.tensor_sub(comp(SPX, j), comp(W, j), ta2)
                V.tensor_scalar(SPX, SPX, 0.0, None, OP.max)  # relu (4x wide)

                Cc = SPX[:, 0 : 3 * B]
                Pp = SPX[:, 3 * B : 6 * B]

                # ---- cosine pieces: W9 = [p*c | p^2 | c^2], pq = sums
                V.tensor_mul(W9[:, 0 : 3 * B], Pp, Cc)
                A.square(W9[:, 3 * B : 6 * B], Pp)
                A.square(W9[:, 6 * B : 9 * B], Cc)
                pq = w3("pq")
                pq3 = pq.rearrange("p (u b) -> p u b", b=B)
                W9v = W9.rearrange("p (u b) -> p u b", b=B)
                V.tensor_add(pq3, W9v[:, 0:9:3, :], W9v[:, 1:9:3, :])
                V.tensor_add(pq3, pq3, W9v[:, 2:9:3, :])
                pc = pq[:, 0:B]
                pps = pq[:, B : 2 * B]
                ccs = pq[:, 2 * B : 3 * B]

                # ---- y = max(p + c, eps); l = y ln y
                Y = YL[:, 0 : 3 * B]
                LNp = YL[:, 3 * B : 6 * B]
                V.tensor_add(Y, Pp, Cc)
                V.tensor_scalar(Y, Y, float(z_eps), None, OP.max)
                A.activation(LNp, Y, AF.Ln)
                V.tensor_mul(LNp, Y, LNp)
                # zq = [zs | szl] via strided pair-sums over YL
                zq = w2("zq", bufs=1)
                zq2 = zq.rearrange("p (u b) -> p u b", b=B)
                YLv = YL.rearrange("p (u b) -> p u b", b=B)
                V.tensor_add(zq2, YLv[:, 0:6:3, :], YLv[:, 1:6:3, :])
                V.tensor_add(zq2, zq2, YLv[:, 2:6:3, :])
                zs = zq[:, 0:B]
                szl = zq[:, B : 2 * B]

                # ---- entropy = ln zs - szl/zs ; cos = 0.1 + pc/sqrt(pps*ccs)
                lzs = pl("lzs")
                izs = pl("izs")
                ent = pl("ent")
                nn = pl("nn")
                irt = pl("irt")
                ie = pl("ie")
                sc = pl("sc")
                V.tensor_mul(nn, pps, ccs)
                # ACT ops grouped by table: {Ln x3} then {Exp x2} then Ln/Exp
                A.activation(lzs, zs, AF.Ln)
                A.activation(irt, nn, AF.Ln)
                A.activation(izs, lzs, AF.Exp, scale=-1.0)  # 1/zs
                A.activation(irt, irt, AF.Exp, scale=-0.5)  # 1/sqrt(nn)
                V.tensor_mul(szl, szl, izs)
                V.tensor_sub(ent, lzs, szl)
                V.tensor_mul(pc, pc, irt)
                V.tensor_scalar(pc, pc, 0.1, None, OP.add)  # cos (4x)
                A.activation(ent, ent, AF.Ln)
                A.activation(ie, ent, AF.Exp, scale=-1.0)  # 1/ent
                V.tensor_mul(sc, pc, ie)  # cos/ent (x21 folded into final op)

                # ---- alpha = max(21 * (cos/ent) * (b*c + (1-b)*p), 0.001)
                # alpha0 = b*c + (1-b)*p as one custom op; b planes need a
                # single per-partition scalar each, so do it per component.
                for i in range(3):
                    V._custom_dve(
                        ops["aff2"],
                        out=p3(Y, i),
                        in0=p3(Cc, i),
                        in1=p3(Pp, i),
                        s0=sv(9 + i),
                        s1=sv(12 + i),
                    )
                for i in range(3):
                    V.tensor_mul(p3(Y, i), p3(Y, i), sc)
                V.tensor_scalar(ot[:], Y, float(scale_factor), 0.001, OP.mult, OP.max)

                nc.sync.dma_start(out_v[t], ot[:])

    nc.compile()
    return nc


def _get_program(z_eps: float, scale_factor: float, T: int):
    key = (round(z_eps, 9), round(scale_factor, 9), T)
    if key not in _PROG_CACHE:
        _PROG_CACHE[key] = _build_program(z_eps, scale_factor, T)
    return _PROG_CACHE[key]


# --------------------------------------------------------------------------
# Host-side pack/unpack
# --------------------------------------------------------------------------
def _pack_core(ch, pp, rels, t15, k, e, T):
    """Returns (xin_f16, scl_f32, order, gi, valid)."""
    sl = slice(k * e, (k + 1) * e)
    r = rels[sl]
    order = np.argsort(r, kind="stable")
    counts = np.bincount(r, minlength=N_RELS)
    nch = (counts + B - 1) // B
    cell_rel = np.repeat(np.arange(N_RELS, dtype=np.int64), nch)
    within = np.concatenate([np.arange(n, dtype=np.int64) * B for n in nch])
    ncells = cell_rel.shape[0]
    ncap = T * PDIM
    assert ncells <= ncap, (ncells, ncap)
    rel_starts = np.concatenate([[0], np.cumsum(counts)[:-1]])
    cell_start = rel_starts[cell_rel] + within
    cell_len = np.minimum(counts[cell_rel] - within, B)
    pad = ncap - ncells
    if pad:
        cell_rel = np.concatenate([cell_rel, np.zeros(pad, np.int64)])
        cell_start = np.concatenate([cell_start, np.zeros(pad, np.int64)])
        cell_len = np.concatenate([cell_len, np.zeros(pad, np.int64)])
    gi = cell_start[:, None] + np.arange(B, dtype=np.int64)[None, :]
    valid = np.arange(B, dtype=np.int64)[None, :] < cell_len[:, None]
    gi = np.where(valid, gi, 0)

    def pack(a):
        s = np.ascontiguousarray(a[sl], dtype=np.float32)[order]  # [e, 3]
        cells = s[gi]  # [ncap, B, 3]
        cells *= valid[..., None]
        return cells.transpose(0, 2, 1).reshape(T, PDIM, 3 * B).astype(np.float16)

    xin = np.concatenate([pack(ch), pack(pp)], axis=2).reshape(-1)
    scl = np.zeros((ncap, 16), dtype=np.float32)
    scl[:, :15] = t15[cell_rel]
    return xin, scl.reshape(-1), order, gi, valid


def _unpack_core(out_f16, order, gi, valid, e, T):
    cells = out_f16.reshape(T * PDIM, 3, B).transpose(0, 2, 1).astype(np.float32)
    res_sorted = np.empty((e, 3), dtype=np.float32)
    res_sorted[gi[valid]] = cells[valid]
    res = np.empty((e, 3), dtype=np.float32)
    res[order] = res_sorted
    return res


def _run(inputs: dict, trace: bool = False):
    ch = np.asarray(inputs["child_probs"], dtype=np.float32)
    pp = np.asarray(inputs["prnt_probs"], dtype=np.float32)
    M = np.asarray(inputs["M"], dtype=np.float32)
    beta = np.asarray(inputs["beta"], dtype=np.float32)
    rels = np.asarray(inputs["rels"]).astype(np.int64)
    z_eps = float(np.asarray(inputs["z_epsilon"]))
    sf = float(np.asarray(inputs["scale_factor"]))

    n = rels.shape[0]
    assert n % N_CORES == 0
    e = n // N_CORES

    t15 = np.concatenate(
        [M.reshape(N_RELS, 9), beta, 1.0 - beta], axis=1
    ).astype(np.float32)

    max_cells = 0
    for k in range(N_CORES):
        counts = np.bincount(rels[k * e : (k + 1) * e], minlength=N_RELS)
        max_cells = max(max_cells, int(((counts + B - 1) // B).sum()))
    T = max(1, -(-max_cells // PDIM))

    packs = [_pack_core(ch, pp, rels, t15, k, e, T) for k in range(N_CORES)]
    nc = _get_program(z_eps, sf, T)
    in_maps = [{"xin": p[0], "scl": p[1]} for p in packs]
    res = run_bass_kernel_spmd(nc, in_maps, core_ids=list(range(N_CORES)), trace=trace)
    outs = [
        _unpack_core(res.results[k]["alpha"], packs[k][2], packs[k][3], packs[k][4], e, T)
        for k in range(N_CORES)
    ]
    return np.concatenate(outs, axis=0), res


def kernel(**inputs) -> np.ndarray:
    out, _ = _run(inputs)
    return out


def kernel_traced(**inputs):
    """Returns (output, BassKernelResults-with-profile) for test harnesses."""
    return _run(inputs, trace=True)


# revision 19
# speedup vs baseline: 1.0129x; 1.0129x over previous
"""AlphaModel (relation-gated message passing) Trainium2 kernel, v3.

Strategy (pure data parallel, per sharding hint):
  - Shard the 8M edges across 8 NeuronCores (1M each).
  - Host SORTS each core's edges by relation id and packs them into
    (tile, partition) cells so that every SBUF partition row processes
    edges of a single relation.  M[rel] / beta[rel] then enter the
    device as per-partition scalar vectors [128,1] (fp32), so the
    relation-gated matvec and beta-mix become tensor_scalar ops (DVE
    2x/4x perf modes) instead of streamed per-edge tables.
  - fp16 end-to-end on-chip (2e-2 rel tolerance).  tensor_tensor ops
    hit the 2x_1p DVE mode, tensor_scalar the 4x_2p mode.
  - The c- and p-branch sparsemaxes are FUSED: both live in one wide
    [128, 6B] tile ([c0 c1 c2 | p0 p1 p2]) and the whole max/min/sum
    tree + tau runs as [128, 2, B] strided ops covering both branches.
  - Activation-table thrash eliminated: only Ln / Square / one Exp on
    ACT (grouped), reciprocals on the DVE (nc.vector.reciprocal).
  - sparsemax (d=3) via simplex projection:
      tau = max(mx-1, (sm-mn-1)/2, (sm-1)/3);  out = relu(x - tau)
    The second sparsemax application in the reference is an exact no-op
    (idempotent projection) and is skipped.

Output: alpha [8M, 3] float32 (device emits fp16, host widens).
"""

import sys

if "/opt/trn_rl_repo" not in sys.path:
    sys.path.insert(0, "/opt/trn_rl_repo")

import numpy as np

import concourse.bacc as bacc
import concourse.mybir as mybir
from concourse.bass_utils import run_bass_kernel_spmd
from concourse.tile import TileContext

N_CORES = 8
PDIM = 128
N_RELS = 64
B = 1024  # edges per (tile, partition) cell

AF = mybir.ActivationFunctionType
OP = mybir.AluOpType
F16 = mybir.dt.float16
F32 = mybir.dt.float32

# --------------------------------------------------------------------------
# Custom fused DVE op (same registration machinery as production dve_ops).
# --------------------------------------------------------------------------
_OPS_CACHE: dict = {}


def _custom_ops():
    if _OPS_CACHE:
        return _OPS_CACHE
    from concourse import dve_ops
    from concourse.dve_ops import DveOp, OPS, _SUB_OPCODE_FOR_NAME
    from concourse.dve_spec import (
        C0,
        C1,
        One,
        Spec,
        Src0,
        Src1,
        _has_src1,
        lower,
        maxx,
    )
    from concourse.dve_uop import DveOpSpec

    existing = {op.name: op for op in OPS}

    def mk(key, name, body):
        if name in existing:
            _OPS_CACHE[key] = existing[name]
            return
        if name not in _SUB_OPCODE_FOR_NAME:
            row = max(_SUB_OPCODE_FOR_NAME.values()) + 1
            assert row < 0x20, "custom DVE opcode rows exhausted"
            _SUB_OPCODE_FOR_NAME[name] = row
        spec = Spec(body=body)
        shas = {}
        for ver in ("v3", "v4"):
            uops = lower(spec, ver=ver)
            s = DveOpSpec(
                name=name,
                opcode=_SUB_OPCODE_FOR_NAME[name],
                uops=uops,
                rd1_en=_has_src1(spec),
            )
            shas[ver] = s.sha(ver)
        op = DveOp(name, spec, subdim=False, uops_sha=shas)
        OPS.append(op)
        dve_ops.CUSTOM_DVE_SPECS[name] = spec
        _OPS_CACHE[key] = op

    # tau candidates: max((sm - mn - 1)*0.5, (sm - 1)/3);  in0=sm, in1=mn
    mk("tau_a", "ANT_TAU_A", maxx((Src0 - Src1 - One) * C0, (Src0 - One) * C1))
    # alpha0 = b*c + (1-b)*p with per-partition scalar APs s0=b, s1=(1-b)
    mk("aff2", "ANT_AFF2", Src0 * C0 + Src1 * C1)
    return _OPS_CACHE


# --------------------------------------------------------------------------
# Bass program
# --------------------------------------------------------------------------
_PROG_CACHE: dict = {}


def _build_program(z_eps: float, scale_factor: float, T: int):
    ops = _custom_ops()
    nc = bacc.Bacc(
        "TRN2",
        target_bir_lowering=False,
        num_devices=N_CORES,
        dynamic_dma_scratch_size=8192,
    )

    # Input streams per tile/partition: child and parent planes, fp16
    xin_d = nc.dram_tensor("xin", [T * PDIM * 3 * B], F16, kind="ExternalInput")
    xpp_d = nc.dram_tensor("xpp", [T * PDIM * 3 * B], F16, kind="ExternalInput")
    # Per (tile, partition) scalars: M00..M22, b0..b2, (1-b)0..(1-b)2, pad
    scl_d = nc.dram_tensor("scl", [T * PDIM * 16], F32, kind="ExternalInput")
    out_d = nc.dram_tensor("alpha", [T * PDIM * 3 * B], F16, kind="ExternalOutput")

    xin_v = xin_d[:].rearrange("(t p c) -> t p c", t=T, p=PDIM)
    xpp_v = xpp_d[:].rearrange("(t p c) -> t p c", t=T, p=PDIM)
    scl_v = scl_d[:].rearrange("(t p c) -> t p c", t=T, p=PDIM)
    out_v = out_d[:].rearrange("(t p c) -> t p c", t=T, p=PDIM)

    V = nc.vector
    A = nc.scalar

    with TileContext(nc) as tc:
        with (
            nc.allow_low_precision(reason="fp16 pipeline; 2e-2 rel tolerance"),
            tc.tile_pool(name="io", bufs=2) as iop,
            tc.tile_pool(name="scr", bufs=2) as sp,
        ):
            for t in range(T):
                xin = iop.tile([PDIM, 3 * B], F16, tag="xin", name=f"xin{t}")
                scl = iop.tile([PDIM, 16], F32, tag="scl", name=f"scl{t}")
                ot = iop.tile([PDIM, 3 * B], F16, tag="ot", name=f"ot{t}")
                nc.sync.dma_start(xin[:], xin_v[t])
                nc.sync.dma_start(scl[:], scl_v[t])

                X = xin[:, 0 : 3 * B]  # child planes

                def w6(tag, bufs=2):
                    return sp.tile(
                        [PDIM, 6 * B], F16, tag=tag, name=f"{tag}{t}", bufs=bufs
                    )[:]

                def w3(tag, bufs=1):
                    return sp.tile(
                        [PDIM, 3 * B], F16, tag=tag, name=f"{tag}{t}", bufs=bufs
                    )[:]

                def w2(tag, bufs=2):
                    return sp.tile(
                        [PDIM, 2 * B], F16, tag=tag, name=f"{tag}{t}", bufs=bufs
                    )[:]

                def pl(tag, bufs=1):
                    return sp.tile(
                        [PDIM, B], F16, tag=tag, name=f"{tag}{t}", bufs=bufs
                    )[:]

                def p3(x, i):
                    return x[:, i * B : (i + 1) * B]

                def sv(j):
                    return scl[:, j : j + 1]

                W = w6("W")  # [c_raw | p_raw]
                nc.sync.dma_start(W[:, 3 * B : 6 * B], xpp_v[t])  # p_raw
                SPX = w6("SPX")  # [c | p] sparsemax outputs
                YL = w6("YL", bufs=1)  # [y | y ln y]
                W9 = sp.tile([PDIM, 9 * B], F16, tag="W9", name=f"W9{t}", bufs=1)[:]

                CRw = W[:, 0 : 3 * B]
                # ---- c_raw_i = sum_j M_ij * ch_j: the 9 products run on the
                # ACT engine (Copy with per-partition scale) into W9 scratch;
                # the DVE only does the 6 accumulating adds.
                for i in range(3):
                    for j in range(3):
                        A.mul(p3(W9, 3 * i + j), p3(X, j), sv(3 * i + j))
                for i in range(3):
                    V.tensor_add(p3(CRw, i), p3(W9, 3 * i), p3(W9, 3 * i + 1))
                    V.tensor_add(p3(CRw, i), p3(CRw, i), p3(W9, 3 * i + 2))

                # ---- fused two-branch sparsemax: SPX = relu(W - tau)
                # strided views: component j of both branches = [128, 2, B]
                Wv = W.rearrange("p (u b) -> p u b", b=B)  # u = 6

                def comp(x6, j):
                    v = x6.rearrange("p (u b) -> p u b", b=B)
                    return v[:, j : j + 4 : 3, :]  # planes {j, j+3}

                mx = w2("mx")
                mn = w2("mn")
                sm = w2("sm")
                ta = w2("ta")
                mx2 = mx.rearrange("p (u b) -> p u b", b=B)
                mn2 = mn.rearrange("p (u b) -> p u b", b=B)
                sm2 = sm.rearrange("p (u b) -> p u b", b=B)
                ta2 = ta.rearrange("p (u b) -> p u b", b=B)
                x0, x1, x2 = comp(W, 0), comp(W, 1), comp(W, 2)
                V.tensor_max(mx2, x0, x1)
                V.tensor_tensor(mn2, x0, x1, OP.min)
                V.tensor_add(sm2, x0, x1)
                V.tensor_max(mx2, mx2, x2)
                V.tensor_tensor(mn2, mn2, x2, OP.min)
                V.tensor_add(sm2, sm2, x2)
                V._custom_dve(ops["tau_a"], out=ta, in0=sm, in1=mn, s0=0.5, s1=1.0 / 3.0)
                V.tensor_scalar(mx, mx, -1.0, None, OP.add)  # mx-1 (4x)
                V.tensor_max(ta, ta, mx)  # tau for both branches
                for j in range(3):
                    V.tensor_sub(comp(SPX, j), comp(W, j), ta2)
                V.tensor_scalar(SPX, SPX, 0.0, None, OP.max)  # relu (4x wide)

                Cc = SPX[:, 0 : 3 * B]
                Pp = SPX[:, 3 * B : 6 * B]

                # ---- cosine pieces: W9 = [p*c | p^2 | c^2], pq = sums
                V.tensor_mul(W9[:, 0 : 3 * B], Pp, Cc)
                A.square(W9[:, 3 * B : 6 * B], Pp)
                A.square(W9[:, 6 * B : 9 * B], Cc)
                pq = w3("pq")
                pq3 = pq.rearrange("p (u b) -> p u b", b=B)
                W9v = W9.rearrange("p (u b) -> p u b", b=B)
                V.tensor_add(pq3, W9v[:, 0:9:3, :], W9v[:, 1:9:3, :])
                V.tensor_add(pq3, pq3, W9v[:, 2:9:3, :])
                pc = pq[:, 0:B]
                pps = pq[:, B : 2 * B]
                ccs = pq[:, 2 * B : 3 * B]

                # ---- y = max(p + c, eps); l = y ln y
                Y = YL[:, 0 : 3 * B]
                LNp = YL[:, 3 * B : 6 * B]
                V.tensor_add(Y, Pp, Cc)
                V.tensor_scalar(Y, Y, float(z_eps), None, OP.max)
                A.activation(LNp, Y, AF.Ln)
                V.tensor_mul(LNp, Y, LNp)
                # zq = [zs | szl] via strided pair-sums over YL
                zq = w2("zq", bufs=1)
                zq2 = zq.rearrange("p (u b) -> p u b", b=B)
                YLv = YL.rearrange("p (u b) -> p u b", b=B)
                V.tensor_add(zq2, YLv[:, 0:6:3, :], YLv[:, 1:6:3, :])
                V.tensor_add(zq2, zq2, YLv[:, 2:6:3, :])
                zs = zq[:, 0:B]
                szl = zq[:, B : 2 * B]

                # ---- entropy = ln zs - szl/zs ; cos = 0.1 + pc/sqrt(pps*ccs)
                # mx/mn/sm are dead after tau; reuse their (double-
                # buffered) storage for the scalar chain to keep tiles
                # pipelining across iterations.
                lzs = mx[:, 0:B]
                izs = mx[:, B : 2 * B]
                nn = mn[:, 0:B]
                irt = mn[:, B : 2 * B]
                sc = sm[:, 0:B]
                ent = sp.tile([PDIM, B], F32, tag="ent", name=f"ent{t}", bufs=2)[:]
                ie = sp.tile([PDIM, B], F32, tag="ie", name=f"ie{t}", bufs=2)[:]
                V.tensor_mul(nn, pps, ccs)
                # ACT ops grouped by table: {Ln x3} then {Exp x2} then Ln/Exp
                A.activation(lzs, zs, AF.Ln)
                A.activation(irt, nn, AF.Ln)
                A.activation(izs, lzs, AF.Exp, scale=-1.0)  # 1/zs
                A.activation(irt, irt, AF.Exp, scale=-0.5)  # 1/sqrt(nn)
                V.tensor_mul(szl, szl, izs)
                V.tensor_sub(ent, lzs, szl)  # fp32
                V.reciprocal_approx_fast(ie, ent)  # 1/ent (fp32, DVE)
                V.tensor_mul(pc, pc, irt)
                V.tensor_scalar(pc, pc, 0.1, None, OP.add)  # cos (4x)
                V.tensor_mul(sc, pc, ie)  # cos/ent (x21 folded into final op)

                # ---- alpha = max(21 * (cos/ent) * (b*c + (1-b)*p), 0.001)
                # alpha0 = b*c + (1-b)*p as one custom op; b planes need a
                # single per-partition scalar each, so do it per component.
                for i in range(3):
                    V._custom_dve(
                        ops["aff2"],
                        out=p3(Y, i),
                        in0=p3(Cc, i),
                        in1=p3(Pp, i),
                        s0=sv(9 + i),
                        s1=sv(12 + i),
                    )
                for i in range(3):
                    V.tensor_mul(p3(Y, i), p3(Y, i), sc)
                V.tensor_scalar(ot[:], Y, float(scale_factor), 0.001, OP.mult, OP.max)

                nc.sync.dma_start(out_v[t], ot[:])

    nc.compile()
    return nc


def _get_program(z_eps: float, scale_factor: float, T: int):
    key = (round(z_eps, 9), round(scale_factor, 9), T)
    if key not in _PROG_CACHE:
        _PROG_CACHE[key] = _build_program(z_eps, scale_factor, T)
    return _PROG_CACHE[key]


# --------------------------------------------------------------------------
# Host-side pack/unpack
# --------------------------------------------------------------------------
def _pack_core(ch, pp, rels, t15, k, e, T):
    """Returns (xch_f16, xpp_f16, scl_f32, order, gi, valid)."""
    sl = slice(k * e, (k + 1) * e)
    r = rels[sl]
    order = np.argsort(r, kind="stable")
    counts = np.bincount(r, minlength=N_RELS)
    nch = (counts + B - 1) // B
    cell_rel = np.repeat(np.arange(N_RELS, dtype=np.int64), nch)
    within = np.concatenate([np.arange(n, dtype=np.int64) * B for n in nch])
    ncells = cell_rel.shape[0]
    ncap = T * PDIM
    assert ncells <= ncap, (ncells, ncap)
    rel_starts = np.concatenate([[0], np.cumsum(counts)[:-1]])
    cell_start = rel_starts[cell_rel] + within
    cell_len = np.minimum(counts[cell_rel] - within, B)
    pad = ncap - ncells
    if pad:
        cell_rel = np.concatenate([cell_rel, np.zeros(pad, np.int64)])
        cell_start = np.concatenate([cell_start, np.zeros(pad, np.int64)])
        cell_len = np.concatenate([cell_len, np.zeros(pad, np.int64)])
    gi = cell_start[:, None] + np.arange(B, dtype=np.int64)[None, :]
    valid = np.arange(B, dtype=np.int64)[None, :] < cell_len[:, None]
    gi = np.where(valid, gi, 0)

    def pack(a):
        s = np.ascontiguousarray(a[sl], dtype=np.float32)[order]  # [e, 3]
        cells = s[gi]  # [ncap, B, 3]
        cells *= valid[..., None]
        return cells.transpose(0, 2, 1).reshape(T, PDIM, 3 * B).astype(np.float16)

    scl = np.zeros((ncap, 16), dtype=np.float32)
    scl[:, :15] = t15[cell_rel]
    return pack(ch).reshape(-1), pack(pp).reshape(-1), scl.reshape(-1), order, gi, valid


def _unpack_core(out_f16, order, gi, valid, e, T):
    cells = out_f16.reshape(T * PDIM, 3, B).transpose(0, 2, 1).astype(np.float32)
    res_sorted = np.empty((e, 3), dtype=np.float32)
    res_sorted[gi[valid]] = cells[valid]
    res = np.empty((e, 3), dtype=np.float32)
    res[order] = res_sorted
    return res


def _run(inputs: dict, trace: bool = False):
    ch = np.asarray(inputs["child_probs"], dtype=np.float32)
    pp = np.asarray(inputs["prnt_probs"], dtype=np.float32)
    M = np.asarray(inputs["M"], dtype=np.float32)
    beta = np.asarray(inputs["beta"], dtype=np.float32)
    rels = np.asarray(inputs["rels"]).astype(np.int64)
    z_eps = float(np.asarray(inputs["z_epsilon"]))
    sf = float(np.asarray(inputs["scale_factor"]))

    n = rels.shape[0]
    assert n % N_CORES == 0
    e = n // N_CORES

    t15 = np.concatenate(
        [M.reshape(N_RELS, 9), beta, 1.0 - beta], axis=1
    ).astype(np.float32)

    max_cells = 0
    for k in range(N_CORES):
        counts = np.bincount(rels[k * e : (k + 1) * e], minlength=N_RELS)
        max_cells = max(max_cells, int(((counts + B - 1) // B).sum()))
    T = max(1, -(-max_cells // PDIM))

    packs = [_pack_core(ch, pp, rels, t15, k, e, T) for k in range(N_CORES)]
    nc = _get_program(z_eps, sf, T)
    in_maps = [{"xin": p[0], "xpp": p[1], "scl": p[2]} for p in packs]
    res = run_bass_kernel_spmd(nc, in_maps, core_ids=list(range(N_CORES)), trace=trace)
    outs = [
        _unpack_core(res.results[k]["alpha"], packs[k][3], packs[k][4], packs[k][5], e, T)
        for k in range(N_CORES)
    ]
    return np.concatenate(outs, axis=0), res


def kernel(**inputs) -> np.ndarray:
    out, _ = _run(inputs)
    return out


def kernel_traced(**inputs):
    """Returns (output, BassKernelResults-with-profile) for test harnesses."""
    return _run(inputs, trace=True)
